# revision 12
# baseline (speedup 1.0000x reference)
"""Trainium2 Bass kernel for the BinaryClassificationLTC problem.

Data-parallel over batch across 8 NeuronCores. Each core runs the full
LTC scan for its 128-row batch shard:
  phase 1: sensory synapse sums (w_num_s/w_den_s) for all T steps,
           computed with PE broadcast-matmuls + big sigmoid ACTs,
           stored in SBUF in a [num|den, t, b] layout.
  phase 2: hardware For_i loop over T steps x 6 ODE unfolds.
           Per unfold: 'broadcast' matmuls build the sigmoid arguments
           sigma*(v - mu) for the ~50% active (i,u) synapse pairs
           (bias folded in via a ones-row), one big Sigmoid ACT over
           PSUM, 'reduce' matmuls contract the masked conductances to
           num/den contributions, and a short DVE tail updates v.
  phase 3: FC head (two matmuls + relu + sigmoid).
"""

import sys

if "/opt/trn_rl_repo" not in sys.path:
    sys.path.insert(0, "/opt/trn_rl_repo")

import numpy as np

import concourse.bass as bass
import concourse.mybir as mybir
import concourse.tile as tile
from concourse import bacc
from concourse.alu_op_type import AluOpType
from concourse.bass import ds
from concourse.bass_utils import run_bass_kernel_spmd

F32 = mybir.dt.float32
AF = mybir.ActivationFunctionType
ODE_UNFOLDS = 6
EPS = 1e-8
N_CORES = 8
P = 128


def _ceil_div(a, b):
    return (a + b - 1) // b


def _pack(inputs):
    """Host-side packing of all LTC parameters into matmul operands."""
    U = inputs["gleak"].shape[0]
    D = inputs["input_w"].shape[0]
    f = lambda k: np.asarray(inputs[k], np.float32)

    def pack_side(mask, sigma, mu, w, erev, in_scale, in_bias):
        # active (j, u) pairs; j indexes the presynaptic axis (i or d)
        jj, uu = np.nonzero(np.asarray(mask) != 0)
        n = len(jj)
        C = max(1, _ceil_div(n, P))
        K = C * P
        ss = np.zeros((U + 1, K), np.float32)   # bcast lhsT (row U = ones-row coeff)
        w2 = np.zeros((P, K), np.float32)       # reduce lhsT, chunk c at cols [c*P,(c+1)*P)
        sg = np.asarray(sigma, np.float64)
        mm = np.asarray(mu, np.float64)
        ww = np.asarray(w, np.float64) * np.asarray(mask != 0, np.float64)
        ee = np.asarray(erev, np.float64)
        isc = np.asarray(in_scale, np.float64)
        ibi = np.asarray(in_bias, np.float64)
        for k in range(n):
            j, u = jj[k], uu[k]
            c, r = k // P, k % P
            # arg = sigma*(in_scale*x + in_bias - mu)
            ss[j, k] = sg[j, u] * isc[j]
            ss[U, k] = sg[j, u] * (ibi[j] - mm[j, u])
            w2[r, c * P + u] = ww[j, u] * ee[j, u]
            w2[r, c * P + U + u] = ww[j, u]
        return ss, w2, C

    ones_d = np.ones((D,), np.float64)
    ss_s, w2_s, C_s = pack_side(
        inputs["sensory_mask"], f("sensory_sigma"), f("sensory_mu"),
        f("sensory_w"), f("sensory_erev"), f("input_w"), f("input_b"))
    ss_r, w2_r, C_r = pack_side(
        inputs["mask"], f("sigma"), f("mu"), f("w"), f("erev"),
        ones_d * 0 + 1.0, ones_d * 0.0)

    cm_t = f("cm") * ODE_UNFOLDS
    gleak, vleak = f("gleak"), f("vleak")
    cmt = np.zeros((P, 1), np.float32)
    cmt[:U, 0] = cm_t
    cns = np.zeros((P, 1), np.float32)
    cns[:U, 0] = gleak * vleak          # added to num
    cns[U:2 * U, 0] = cm_t + gleak + EPS  # added to den

    # FC head with output affine folded in
    ow = float(np.asarray(inputs["output_w"]).ravel()[0])
    ob = float(np.asarray(inputs["output_b"]).ravel()[0])
    fc1_w = f("fc1_w")  # [H, T]
    H, T = fc1_w.shape
    fc1_wp = fc1_w * ow
    fc1_bp = f("fc1_b") + ob * fc1_w.sum(axis=1)
    TQ = _ceil_div(T, P)
    f1 = np.zeros((P, TQ * P), np.float32)
    for q in range(TQ):
        t0, t1 = q * P, min((q + 1) * P, T)
        f1[0:t1 - t0, q * P:q * P + H] = fc1_wp[:, t0:t1].T
    f1b = fc1_bp.reshape(H, 1).astype(np.float32)
    g2 = f("fc2_w").reshape(1, H).T.copy()  # [H, 1]
    f2b = np.asarray(inputs["fc2_b"], np.float32).reshape(1, 1)

    return dict(ss_s=ss_s, w2_s=w2_s, C_s=C_s, ss_r=ss_r, w2_r=w2_r, C_r=C_r,
                cmt=cmt, cns=cns, f1=f1, f1b=f1b, g2=g2, f2b=f2b,
                idm=np.eye(P, dtype=np.float32),
                U=U, D=D, T=T, H=H, TQ=TQ)


_PROG_CACHE = {}


def _build(C_s, C_r, T, U, H, TQ, Bc, debug_dump=False):
    """Build the SPMD Bass program (identical on all cores)."""
    key = (C_s, C_r, T, U, H, TQ, Bc, debug_dump)
    if key in _PROG_CACHE:
        return _PROG_CACHE[key]

    nc = bacc.Bacc("TRN2", target_bir_lowering=False, debug=False,
                   num_devices=N_CORES)
    x_d = nc.dram_tensor("x", [Bc, T, U], F32, kind="ExternalInput")
    ssr_d = nc.dram_tensor("ssr", [U + 1, C_r * P], F32, kind="ExternalInput")
    w2r_d = nc.dram_tensor("w2r", [P, C_r * P], F32, kind="ExternalInput")
    sss_d = nc.dram_tensor("sss", [U + 1, C_s * P], F32, kind="ExternalInput")
    w2s_d = nc.dram_tensor("w2s", [P, C_s * P], F32, kind="ExternalInput")
    cmt_d = nc.dram_tensor("cmt", [P, 1], F32, kind="ExternalInput")
    cns_d = nc.dram_tensor("cns", [P, 1], F32, kind="ExternalInput")
    f1_d = nc.dram_tensor("f1", [P, TQ * P], F32, kind="ExternalInput")
    f1b_d = nc.dram_tensor("f1b", [H, 1], F32, kind="ExternalInput")
    g2_d = nc.dram_tensor("g2", [H, 1], F32, kind="ExternalInput")
    f2b_d = nc.dram_tensor("f2b", [1, 1], F32, kind="ExternalInput")
    idm_d = nc.dram_tensor("idm", [P, P], F32, kind="ExternalInput")
    y_d = nc.dram_tensor("y", [1, Bc], F32, kind="ExternalOutput")
    if debug_dump:
        nsdbg_d = nc.dram_tensor("nsdbg", [P, T * Bc], F32, kind="ExternalOutput")
        seqdbg_d = nc.dram_tensor("seqdbg", [T, Bc], F32, kind="ExternalOutput")
        vdbg_d = nc.dram_tensor("vdbg", [U, Bc], F32, kind="ExternalOutput")

    TG = 4                      # time steps per phase-1 group (N = TG*Bc = 512)
    n_groups = _ceil_div(T, TG)

    with tile.TileContext(nc) as tc:
        with (
            tc.tile_pool(name="consts", bufs=1) as consts,
            tc.tile_pool(name="dram", bufs=1, space="DRAM") as drampool,
            tc.tile_pool(name="work", bufs=1) as work,
        ):
            ssr_t = consts.tile([U + 1, C_r * P], F32)
            w2r_t = consts.tile([P, C_r * P], F32)
            sss_t = consts.tile([U + 1, C_s * P], F32)
            w2s_t = consts.tile([P, C_s * P], F32)
            cmt_t = consts.tile([P, 1], F32)
            cns_t = consts.tile([P, 1], F32)
            f1_t = consts.tile([P, TQ * P], F32)
            f1b_t = consts.tile([H, 1], F32)
            g2_t = consts.tile([H, 1], F32)
            f2b_t = consts.tile([1, 1], F32)
            idm_t = consts.tile([P, P], F32)
            for tl, dr in [(ssr_t, ssr_d), (w2r_t, w2r_d), (sss_t, sss_d),
                           (w2s_t, w2s_d), (cmt_t, cmt_d), (cns_t, cns_d),
                           (f1_t, f1_d), (f1b_t, f1b_d), (g2_t, g2_d),
                           (f2b_t, f2b_d), (idm_t, idm_d)]:
                nc.sync.dma_start(tl[:], dr.ap())

            ns_all = work.tile([P, T * Bc], F32)   # [num|den, (t, b)]
            seq_d = drampool.tile([T, Bc], F32)

            # ---------------- phase 1: sensory sums for all t ----------------
            with (
                tc.tile_pool(name="xe", bufs=3) as xpool,
                tc.tile_pool(name="wacts", bufs=2) as wspool,
                tc.tile_pool(name="bs_ps", bufs=2, space="PSUM") as bspool,
                tc.tile_pool(name="acc_ps", bufs=2, space="PSUM") as accpool,
            ):
                for g in range(n_groups):
                    tg = min(TG, T - g * TG)
                    N = tg * Bc
                    xe = xpool.tile([U + 1, TG * Bc], F32, tag="xe")
                    for tau in range(tg):
                        nc.sync.dma_start(
                            xe[0:U, tau * Bc:(tau + 1) * Bc],
                            x_d.ap()[:, g * TG + tau, :].rearrange("b d -> d b"))
                    nc.gpsimd.memset(xe[U:U + 1, 0:N], 1.0)
                    accs = accpool.tile([P, TG * Bc], F32, tag="accs")
                    for c0 in range(0, C_s, 2):
                        nch = min(2, C_s - c0)
                        bs = bspool.tile([P, 2 * 512], F32, tag="bs")
                        for c in range(c0, c0 + nch):
                            nc.tensor.matmul(
                                bs[:, (c - c0) * 512:(c - c0) * 512 + N],
                                sss_t[0:U + 1, c * P:(c + 1) * P],
                                xe[0:U + 1, 0:N], start=True, stop=True)
                        ws = wspool.tile([P, 2 * 512], F32, tag="ws")
                        if N == 512 and nch == 2:
                            nc.scalar.activation(ws[:, :], bs[:, :], AF.Sigmoid)
                        else:
                            for c in range(c0, c0 + nch):
                                j = (c - c0) * 512
                                nc.scalar.activation(
                                    ws[:, j:j + N], bs[:, j:j + N], AF.Sigmoid)
                        for c in range(c0, c0 + nch):
                            nc.tensor.matmul(
                                accs[:, 0:N],
                                w2s_t[:, c * P:(c + 1) * P],
                                ws[:, (c - c0) * 512:(c - c0) * 512 + N],
                                start=(c == 0), stop=(c == C_s - 1),
                                skip_group_check=True)
                    nc.vector.tensor_scalar_add(
                        ns_all[:, g * TG * Bc:g * TG * Bc + N],
                        accs[:, 0:N], cns_t[:, 0:1])

            if debug_dump:
                nc.sync.dma_start(nsdbg_d.ap(), ns_all[:, :])

            # ---------------- phase 2: the scan ----------------
            v2 = work.tile([U + 1, Bc], F32)
            wact = work.tile([P, C_r * P], F32)
            rden = work.tile([P, Bc], F32)
            nc.vector.memset(v2[0:U, :], 0.0)
            nc.vector.memset(v2[U:U + 1, :], 1.0)

            with (
                tc.tile_pool(name="b_ps", bufs=1, space="PSUM") as bp2,
                tc.tile_pool(name="a_ps", bufs=1, space="PSUM") as ap2,
            ):
                B_ps = bp2.tile([P, C_r * P], F32)
                acc = ap2.tile([P, Bc], F32)
                h_splits = [0, C_r // 2, C_r] if C_r >= 2 else [0, C_r]

                with tc.For_i(0, T, 1) as iv:
                    for n in range(ODE_UNFOLDS):
                        for c in range(C_r):
                            nc.tensor.matmul(
                                B_ps[:, c * P:(c + 1) * P],
                                ssr_t[0:U + 1, c * P:(c + 1) * P],
                                v2[0:U + 1, :], start=True, stop=True,
                                skip_group_check=True)
                        for hi in range(len(h_splits) - 1):
                            a, b = h_splits[hi] * P, h_splits[hi + 1] * P
                            nc.scalar.activation(
                                wact[:, a:b], B_ps[:, a:b], AF.Sigmoid)
                        # acc = ns_t + sum_c W2_c^T wact_c   (PSUM group)
                        nc.tensor.matmul(
                            acc[:, :], idm_t[:, :], ns_all[:, ds(iv * Bc, Bc)],
                            start=True, stop=False, skip_group_check=True)
                        for c in range(C_r):
                            nc.tensor.matmul(
                                acc[:, :], w2r_t[:, c * P:(c + 1) * P],
                                wact[:, c * P:(c + 1) * P],
                                start=False, stop=(c == C_r - 1),
                                skip_group_check=True)
                        # acc[num] += cm_t * v   (in-place, PSUM src+dst)
                        nc.vector.scalar_tensor_tensor(
                            acc[0:U, :], v2[0:U, :], cmt_t[0:U, 0:1],
                            acc[0:U, :], op0=AluOpType.mult, op1=AluOpType.add)
                        nc.vector.reciprocal(rden[U:2 * U, :], acc[U:2 * U, :])
                        nc.vector.tensor_tensor(
                            v2[0:U, :], acc[0:U, :], rden[U:2 * U, :],
                            op=AluOpType.mult)
                    nc.sync.dma_start(seq_d[ds(iv, 1), :], v2[0:1, :])

            if debug_dump:
                nc.sync.dma_start(vdbg_d.ap(), v2[0:U, :])
                sqd = work.tile([P, Bc], F32)
                for q in range(TQ):
                    t0q, t1q = q * P, min((q + 1) * P, T)
                    nc.sync.dma_start(sqd[0:t1q - t0q, :], seq_d[t0q:t1q, :])
                    nc.sync.dma_start(seqdbg_d.ap()[t0q:t1q, :],
                                      sqd[0:t1q - t0q, :])

            # ---------------- phase 3: FC head ----------------
            with tc.tile_pool(name="fc_ps", bufs=1, space="PSUM") as fcp:
                h_sb = work.tile([H, Bc], F32)
                y_sb = work.tile([1, Bc], F32)
                h_ps = fcp.tile([H, Bc], F32)
                y_ps = fcp.tile([1, Bc], F32)
                sts = []
                for q in range(TQ):
                    st = work.tile([P, Bc], F32)
                    t0q, t1q = q * P, min((q + 1) * P, T)
                    nc.sync.dma_start(st[0:t1q - t0q, :], seq_d[t0q:t1q, :])
                    sts.append((st, t1q - t0q))
                for q, (st, kq) in enumerate(sts):
                    nc.tensor.matmul(h_ps[:, :], f1_t[0:kq, q * P:q * P + H],
                                     st[0:kq, :], start=(q == 0),
                                     stop=(q == TQ - 1), skip_group_check=True)
                nc.scalar.activation(h_sb[:, :], h_ps[:, :], AF.Relu,
                                     bias=f1b_t[:, 0:1])
                nc.tensor.matmul(y_ps[:, :], g2_t[0:H, 0:1], h_sb[:, :],
                                 start=True, stop=True, skip_group_check=True)
                nc.scalar.activation(y_sb[0:1, :], y_ps[0:1, :], AF.Sigmoid,
                                     bias=f2b_t[0:1, 0:1])
                nc.sync.dma_start(y_d.ap(), y_sb[0:1, :])

    nc.compile()
    _PROG_CACHE[key] = nc
    return nc


def _run(inputs, n_cores=N_CORES):
    x = np.ascontiguousarray(np.asarray(inputs["x"], np.float32))
    B, T, D = x.shape
    Bc = B // n_cores
    pk = _pack(inputs)
    nc = _build(pk["C_s"], pk["C_r"], T, pk["U"], pk["H"], pk["TQ"], Bc)

    const_map = {k: np.ascontiguousarray(pk[k]) for k in
                 ["cmt", "cns", "f1", "f1b", "g2", "f2b", "idm"]}
    const_map["ssr"] = np.ascontiguousarray(pk["ss_r"])
    const_map["w2r"] = np.ascontiguousarray(pk["w2_r"])
    const_map["sss"] = np.ascontiguousarray(pk["ss_s"])
    const_map["w2s"] = np.ascontiguousarray(pk["w2_s"])

    in_maps = []
    for c in range(n_cores):
        m = dict(const_map)
        m["x"] = np.ascontiguousarray(x[c * Bc:(c + 1) * Bc])
        in_maps.append(m)

    res = run_bass_kernel_spmd(nc, in_maps, core_ids=list(range(n_cores)))
    y = np.concatenate([res.results[c]["y"][0] for c in range(n_cores)])
    return y.reshape(B, 1).astype(np.float32)


def kernel(**inputs):
    return _run(inputs)


# revision 13
# speedup vs baseline: 42.7886x; 42.7886x over previous
"""Trainium2 Bass kernel for the BinaryClassificationLTC problem.

Data-parallel over batch across 8 NeuronCores. Each core runs the full
LTC scan for its 128-row batch shard:
  phase 1: sensory synapse sums (w_num_s/w_den_s) for all T steps,
           computed with PE broadcast-matmuls + big sigmoid ACTs,
           stored in SBUF in a [num|den, t, b] layout.
  phase 2: hardware For_i loop over T steps x 6 ODE unfolds.
           Per unfold: 'broadcast' matmuls build the sigmoid arguments
           sigma*(v - mu) for the ~50% active (i,u) synapse pairs
           (bias folded in via a ones-row), one big Sigmoid ACT over
           PSUM, 'reduce' matmuls contract the masked conductances to
           num/den contributions, and a short DVE tail updates v.
  phase 3: FC head (two matmuls + relu + sigmoid).
"""

import sys

if "/opt/trn_rl_repo" not in sys.path:
    sys.path.insert(0, "/opt/trn_rl_repo")

import numpy as np

import concourse.bass as bass
import concourse.mybir as mybir
import concourse.tile as tile
from concourse import bacc
from concourse.alu_op_type import AluOpType
from concourse.bass import ds
from concourse.bass_utils import run_bass_kernel_spmd

F32 = mybir.dt.float32
AF = mybir.ActivationFunctionType
ODE_UNFOLDS = 6
EPS = 1e-8
N_CORES = 8
P = 128


def _ceil_div(a, b):
    return (a + b - 1) // b


def _pack(inputs):
    """Host-side packing of all LTC parameters into matmul operands."""
    U = inputs["gleak"].shape[0]
    D = inputs["input_w"].shape[0]
    f = lambda k: np.asarray(inputs[k], np.float32)

    def pack_side(mask, sigma, mu, w, erev, in_scale, in_bias):
        # active (j, u) pairs; j indexes the presynaptic axis (i or d)
        jj, uu = np.nonzero(np.asarray(mask) != 0)
        n = len(jj)
        C = max(1, _ceil_div(n, P))
        K = C * P
        ss = np.zeros((U + 1, K), np.float32)   # bcast lhsT (row U = ones-row coeff)
        w2 = np.zeros((P, K), np.float32)       # reduce lhsT, chunk c at cols [c*P,(c+1)*P)
        sg = np.asarray(sigma, np.float64)
        mm = np.asarray(mu, np.float64)
        ww = np.asarray(w, np.float64) * np.asarray(mask != 0, np.float64)
        ee = np.asarray(erev, np.float64)
        isc = np.asarray(in_scale, np.float64)
        ibi = np.asarray(in_bias, np.float64)
        for k in range(n):
            j, u = jj[k], uu[k]
            c, r = k // P, k % P
            # arg = sigma*(in_scale*x + in_bias - mu)
            ss[j, k] = sg[j, u] * isc[j]
            ss[U, k] = sg[j, u] * (ibi[j] - mm[j, u])
            w2[r, c * P + u] = ww[j, u] * ee[j, u]
            w2[r, c * P + U + u] = ww[j, u]
        return ss, w2, C

    ones_d = np.ones((D,), np.float64)
    ss_s, w2_s, C_s = pack_side(
        inputs["sensory_mask"], f("sensory_sigma"), f("sensory_mu"),
        f("sensory_w"), f("sensory_erev"), f("input_w"), f("input_b"))
    ss_r, w2_r, C_r = pack_side(
        inputs["mask"], f("sigma"), f("mu"), f("w"), f("erev"),
        ones_d * 0 + 1.0, ones_d * 0.0)

    # Error-compensated bf16 split of the recurrent bcast lhsT:
    #   arg = sigma*v - sigma*mu
    #       = sh*(vh+vl) + sl*vh + (-(sm)h - (sm)l)  up to sl*vl (~3e-5)
    # MM-A (K=128): rows [0,64) sh @ i_k on vh-rows, [64,128) sh @ i_k on
    # vl-rows. MM-B (K=66): rows [0,64) sl @ i_k on vh; rows 64/65: ones
    # rows carrying -(sigma*mu) hi/lo.
    import ml_dtypes
    bf = lambda a: np.asarray(a, ml_dtypes.bfloat16).astype(np.float64)
    KR = C_r * P
    sig_row = ss_r[:U, :]                     # sigma at (i_k, k), zero else
    smu_row = -ss_r[U, :]                     # sigma*mu per k
    sh = bf(sig_row)
    sl = sig_row - sh                         # fits bf16 exactly enough
    smh = bf(smu_row)
    sml = smu_row - smh
    ssrA = np.zeros((P, KR), np.float64)
    ssrA[:U, :] = sh
    ssrA[U:2 * U, :] = sh
    ssrB = np.zeros((U + 2, KR), np.float64)
    ssrB[:U, :] = sl
    ssrB[U, :] = -smh
    ssrB[U + 1, :] = -sml
    ssrA = ssrA.astype(ml_dtypes.bfloat16)
    ssrB = ssrB.astype(ml_dtypes.bfloat16)

    cm_t = f("cm") * ODE_UNFOLDS
    gleak, vleak = f("gleak"), f("vleak")
    cmt = np.zeros((P, 1), np.float32)
    cmt[:U, 0] = cm_t
    cns = np.zeros((P, 1), np.float32)
    cns[:U, 0] = gleak * vleak          # added to num
    cns[U:2 * U, 0] = cm_t + gleak + EPS  # added to den

    # FC head with output affine folded in
    ow = float(np.asarray(inputs["output_w"]).ravel()[0])
    ob = float(np.asarray(inputs["output_b"]).ravel()[0])
    fc1_w = f("fc1_w")  # [H, T]
    H, T = fc1_w.shape
    fc1_wp = fc1_w * ow
    fc1_bp = f("fc1_b") + ob * fc1_w.sum(axis=1)
    TQ = _ceil_div(T, P)
    f1 = np.zeros((P, TQ * P), np.float32)
    for q in range(TQ):
        t0, t1 = q * P, min((q + 1) * P, T)
        f1[0:t1 - t0, q * P:q * P + H] = fc1_wp[:, t0:t1].T
    f1b = fc1_bp.reshape(H, 1).astype(np.float32)
    g2 = f("fc2_w").reshape(1, H).T.copy()  # [H, 1]
    f2b = np.asarray(inputs["fc2_b"], np.float32).reshape(1, 1)

    return dict(ss_s=ss_s, w2_s=w2_s, C_s=C_s, ss_r=ss_r, w2_r=w2_r, C_r=C_r,
                cmt=cmt, cns=cns, f1=f1, f1b=f1b, g2=g2, f2b=f2b,
                idm=np.eye(P, dtype=np.float32),
                U=U, D=D, T=T, H=H, TQ=TQ)


_PROG_CACHE = {}


def _build(C_s, C_r, T, U, H, TQ, Bc, debug_dump=False):
    """Build the SPMD Bass program (identical on all cores)."""
    key = (C_s, C_r, T, U, H, TQ, Bc, debug_dump)
    if key in _PROG_CACHE:
        return _PROG_CACHE[key]

    nc = bacc.Bacc("TRN2", target_bir_lowering=False, debug=False,
                   num_devices=N_CORES)
    x_d = nc.dram_tensor("x", [Bc, T, U], F32, kind="ExternalInput")
    ssr_d = nc.dram_tensor("ssr", [U + 1, C_r * P], F32, kind="ExternalInput")
    w2r_d = nc.dram_tensor("w2r", [P, C_r * P], F32, kind="ExternalInput")
    sss_d = nc.dram_tensor("sss", [U + 1, C_s * P], F32, kind="ExternalInput")
    w2s_d = nc.dram_tensor("w2s", [P, C_s * P], F32, kind="ExternalInput")
    cmt_d = nc.dram_tensor("cmt", [P, 1], F32, kind="ExternalInput")
    cns_d = nc.dram_tensor("cns", [P, 1], F32, kind="ExternalInput")
    f1_d = nc.dram_tensor("f1", [P, TQ * P], F32, kind="ExternalInput")
    f1b_d = nc.dram_tensor("f1b", [H, 1], F32, kind="ExternalInput")
    g2_d = nc.dram_tensor("g2", [H, 1], F32, kind="ExternalInput")
    f2b_d = nc.dram_tensor("f2b", [1, 1], F32, kind="ExternalInput")
    idm_d = nc.dram_tensor("idm", [P, P], F32, kind="ExternalInput")
    y_d = nc.dram_tensor("y", [1, Bc], F32, kind="ExternalOutput")
    if debug_dump:
        nsdbg_d = nc.dram_tensor("nsdbg", [P, T * Bc], F32, kind="ExternalOutput")
        seqdbg_d = nc.dram_tensor("seqdbg", [T, Bc], F32, kind="ExternalOutput")
        vdbg_d = nc.dram_tensor("vdbg", [U, Bc], F32, kind="ExternalOutput")

    TG = 4                      # time steps per phase-1 group (N = TG*Bc = 512)
    n_groups = _ceil_div(T, TG)

    with tile.TileContext(nc) as tc:
        with (
            tc.tile_pool(name="consts", bufs=1) as consts,
            tc.tile_pool(name="dram", bufs=1, space="DRAM") as drampool,
            tc.tile_pool(name="work", bufs=1) as work,
        ):
            ssr_t = consts.tile([U + 1, C_r * P], F32)
            w2r_t = consts.tile([P, C_r * P], F32)
            sss_t = consts.tile([U + 1, C_s * P], F32)
            w2s_t = consts.tile([P, C_s * P], F32)
            cmt_t = consts.tile([P, 1], F32)
            cns_t = consts.tile([P, 1], F32)
            f1_t = consts.tile([P, TQ * P], F32)
            f1b_t = consts.tile([H, 1], F32)
            g2_t = consts.tile([H, 1], F32)
            f2b_t = consts.tile([1, 1], F32)
            idm_t = consts.tile([P, P], F32)
            for tl, dr in [(ssr_t, ssr_d), (w2r_t, w2r_d), (sss_t, sss_d),
                           (w2s_t, w2s_d), (cmt_t, cmt_d), (cns_t, cns_d),
                           (f1_t, f1_d), (f1b_t, f1b_d), (g2_t, g2_d),
                           (f2b_t, f2b_d), (idm_t, idm_d)]:
                nc.sync.dma_start(tl[:], dr.ap())

            ns_all = work.tile([P, T * Bc], F32)   # [num|den, (t, b)]
            seq_d = drampool.tile([T, Bc], F32)

            # ---------------- phase 1: sensory sums for all t ----------------
            with (
                tc.tile_pool(name="xe", bufs=3) as xpool,
                tc.tile_pool(name="wacts", bufs=2) as wspool,
                tc.tile_pool(name="bs_ps", bufs=2, space="PSUM") as bspool,
                tc.tile_pool(name="acc_ps", bufs=2, space="PSUM") as accpool,
            ):
                for g in range(n_groups):
                    tg = min(TG, T - g * TG)
                    N = tg * Bc
                    xe = xpool.tile([U + 1, TG * Bc], F32, tag="xe")
                    for tau in range(tg):
                        nc.sync.dma_start(
                            xe[0:U, tau * Bc:(tau + 1) * Bc],
                            x_d.ap()[:, g * TG + tau, :].rearrange("b d -> d b"))
                    nc.gpsimd.memset(xe[U:U + 1, 0:N], 1.0)
                    accs = accpool.tile([P, TG * Bc], F32, tag="accs")
                    for c0 in range(0, C_s, 2):
                        nch = min(2, C_s - c0)
                        bs = bspool.tile([P, 2 * 512], F32, tag="bs")
                        for c in range(c0, c0 + nch):
                            nc.tensor.matmul(
                                bs[:, (c - c0) * 512:(c - c0) * 512 + N],
                                sss_t[0:U + 1, c * P:(c + 1) * P],
                                xe[0:U + 1, 0:N], start=True, stop=True)
                        ws = wspool.tile([P, 2 * 512], F32, tag="ws")
                        if N == 512 and nch == 2:
                            nc.scalar.activation(ws[:, :], bs[:, :], AF.Sigmoid)
                        else:
                            for c in range(c0, c0 + nch):
                                j = (c - c0) * 512
                                nc.scalar.activation(
                                    ws[:, j:j + N], bs[:, j:j + N], AF.Sigmoid)
                        for c in range(c0, c0 + nch):
                            nc.tensor.matmul(
                                accs[:, 0:N],
                                w2s_t[:, c * P:(c + 1) * P],
                                ws[:, (c - c0) * 512:(c - c0) * 512 + N],
                                start=(c == 0), stop=(c == C_s - 1),
                                skip_group_check=True)
                    nc.vector.tensor_scalar_add(
                        ns_all[:, g * TG * Bc:g * TG * Bc + N],
                        accs[:, 0:N], cns_t[:, 0:1])

            if debug_dump:
                nc.sync.dma_start(nsdbg_d.ap(), ns_all[:, :])

            # ---------------- phase 2: the scan ----------------
            v2 = work.tile([U + 1, Bc], F32)
            wact = work.tile([P, C_r * P], F32)
            rden = work.tile([P, Bc], F32)
            nc.vector.memset(v2[0:U, :], 0.0)
            nc.vector.memset(v2[U:U + 1, :], 1.0)

            with (
                tc.tile_pool(name="b_ps", bufs=1, space="PSUM") as bp2,
                tc.tile_pool(name="a_ps", bufs=1, space="PSUM") as ap2,
            ):
                B_ps = bp2.tile([P, C_r * P], F32)
                acc = ap2.tile([P, Bc], F32)
                h_splits = [0, C_r // 2, C_r] if C_r >= 2 else [0, C_r]

                with tc.For_i(0, T, 1) as iv:
                    for n in range(ODE_UNFOLDS):
                        for c in range(C_r):
                            nc.tensor.matmul(
                                B_ps[:, c * P:(c + 1) * P],
                                ssr_t[0:U + 1, c * P:(c + 1) * P],
                                v2[0:U + 1, :], start=True, stop=True,
                                skip_group_check=True)
                        for hi in range(len(h_splits) - 1):
                            a, b = h_splits[hi] * P, h_splits[hi + 1] * P
                            nc.scalar.activation(
                                wact[:, a:b], B_ps[:, a:b], AF.Sigmoid)
                        # acc = ns_t + sum_c W2_c^T wact_c   (PSUM group)
                        nc.tensor.matmul(
                            acc[:, :], idm_t[:, :], ns_all[:, ds(iv * Bc, Bc)],
                            start=True, stop=False, skip_group_check=True)
                        for c in range(C_r):
                            nc.tensor.matmul(
                                acc[:, :], w2r_t[:, c * P:(c + 1) * P],
                                wact[:, c * P:(c + 1) * P],
                                start=False, stop=(c == C_r - 1),
                                skip_group_check=True)
                        # acc[num] += cm_t * v   (in-place, PSUM src+dst)
                        nc.vector.scalar_tensor_tensor(
                            acc[0:U, :], v2[0:U, :], cmt_t[0:U, 0:1],
                            acc[0:U, :], op0=AluOpType.mult, op1=AluOpType.add)
                        nc.vector.reciprocal(rden[U:2 * U, :], acc[U:2 * U, :])
                        nc.vector.tensor_tensor(
                            v2[0:U, :], acc[0:U, :], rden[U:2 * U, :],
                            op=AluOpType.mult)
                    nc.sync.dma_start(seq_d[ds(iv, 1), :], v2[0:1, :])

            if debug_dump:
                nc.sync.dma_start(vdbg_d.ap(), v2[0:U, :])
                sqd = work.tile([P, Bc], F32)
                for q in range(TQ):
                    t0q, t1q = q * P, min((q + 1) * P, T)
                    nc.sync.dma_start(sqd[0:t1q - t0q, :], seq_d[t0q:t1q, :])
                    nc.sync.dma_start(seqdbg_d.ap()[t0q:t1q, :],
                                      sqd[0:t1q - t0q, :])

            # ---------------- phase 3: FC head ----------------
            with tc.tile_pool(name="fc_ps", bufs=1, space="PSUM") as fcp:
                h_sb = work.tile([H, Bc], F32)
                y_sb = work.tile([1, Bc], F32)
                h_ps = fcp.tile([H, Bc], F32)
                y_ps = fcp.tile([1, Bc], F32)
                sts = []
                for q in range(TQ):
                    st = work.tile([P, Bc], F32)
                    t0q, t1q = q * P, min((q + 1) * P, T)
                    nc.sync.dma_start(st[0:t1q - t0q, :], seq_d[t0q:t1q, :])
                    sts.append((st, t1q - t0q))
                for q, (st, kq) in enumerate(sts):
                    nc.tensor.matmul(h_ps[:, :], f1_t[0:kq, q * P:q * P + H],
                                     st[0:kq, :], start=(q == 0),
                                     stop=(q == TQ - 1), skip_group_check=True)
                nc.scalar.activation(h_sb[:, :], h_ps[:, :], AF.Relu,
                                     bias=f1b_t[:, 0:1])
                nc.tensor.matmul(y_ps[:, :], g2_t[0:H, 0:1], h_sb[:, :],
                                 start=True, stop=True, skip_group_check=True)
                nc.scalar.activation(y_sb[0:1, :], y_ps[0:1, :], AF.Sigmoid,
                                     bias=f2b_t[0:1, 0:1])
                nc.sync.dma_start(y_d.ap(), y_sb[0:1, :])

    nc.compile()
    _PROG_CACHE[key] = nc
    return nc


def _run(inputs, n_cores=N_CORES):
    x = np.ascontiguousarray(np.asarray(inputs["x"], np.float32))
    B, T, D = x.shape
    Bc = B // n_cores
    pk = _pack(inputs)
    nc = _build(pk["C_s"], pk["C_r"], T, pk["U"], pk["H"], pk["TQ"], Bc)

    const_map = {k: np.ascontiguousarray(pk[k]) for k in
                 ["cmt", "cns", "f1", "f1b", "g2", "f2b", "idm"]}
    const_map["ssr"] = np.ascontiguousarray(pk["ss_r"])
    const_map["w2r"] = np.ascontiguousarray(pk["w2_r"])
    const_map["sss"] = np.ascontiguousarray(pk["ss_s"])
    const_map["w2s"] = np.ascontiguousarray(pk["w2_s"])

    in_maps = []
    for c in range(n_cores):
        m = dict(const_map)
        m["x"] = np.ascontiguousarray(x[c * Bc:(c + 1) * Bc])
        in_maps.append(m)

    res = run_bass_kernel_spmd(nc, in_maps, core_ids=list(range(n_cores)))
    y = np.concatenate([res.results[c]["y"][0] for c in range(n_cores)])
    return y.reshape(B, 1).astype(np.float32)


def kernel(**inputs):
    return _run(inputs)
